# revision 6
# baseline (speedup 1.0000x reference)
"""Bipartite GNN (factor -> variable) message passing on 8 Trainium2 NeuronCores.

v2: destination-sharded graph parallel, factorized message MLP.
  - relu([x_i, x_j] @ Wm + bm) == relu(yv[s] + zf[r]) with yv = V @ Wm_top + bm
    (own slice, SBUF-resident) and zf = F @ Wm_bot (full table, staged to DRAM).
  - zf rows fetched per edge with dma_gather spread over 4 SWDGE queues
    (2048 idx / call, multi-packet).
  - G^T (slot one-hot, [slot, edge]) built by DMA partition-broadcast of the
    host-precomputed slot stream + one DVE is_equal per 2048-edge batch.
  - G ([edge, slot]) built per 4-chunk group with a 3D broadcast is_equal.
  - msg = relu(G^T.T @ yv_blk + zb) via PE matmuls into [128,512] PSUM groups,
    relu on Act; aggT += msg^T @ G via PE; combine MLP + residual per block.
  - Output slices are disjoint: no collectives.
"""

import numpy as np
import ml_dtypes

BF16 = ml_dtypes.bfloat16
SLOT_INVALID = 255.0

N_VAR, N_FAC, N_EDGE = 100000, 50000, 1000000
N_CORES = 8
CPB = 16  # chunks (of 128 edges) per gather batch -> 2048 edges / batch
D = 128


def _cdiv(a, b):
    return -(-a // b)


# --------------------------------------------------------------------------
# Host-side planning (indices only)
# --------------------------------------------------------------------------

def _make_plan(senders, receivers, n_var, n_fac, n_cores, cpb):
    send = np.asarray(senders).astype(np.int64).ravel()
    recv = np.asarray(receivers).astype(np.int64).ravel()

    # global 128-var blocks, balanced across cores by edge count: round k
    # hands the 8 closest-count blocks to the 8 cores, which minimizes
    # sum_k max_c count so the SPMD per-block chunk padding stays small.
    gblk = _cdiv(n_var, 128)
    nblk = _cdiv(gblk, n_cores)
    gcounts = np.bincount(send >> 7, minlength=gblk)
    order = np.argsort(-gcounts, kind="stable")
    blocks_of_core = np.full((n_cores, nblk), -1, np.int64)
    for k in range(nblk):
        sl = order[k * n_cores : (k + 1) * n_cores]
        blocks_of_core[: len(sl), k] = sl
    owner = np.full(gblk, -1, np.int64)
    kidx = np.full(gblk, -1, np.int64)
    for c in range(n_cores):
        for k in range(nblk):
            g = blocks_of_core[c, k]
            if g >= 0:
                owner[g] = c
                kidx[g] = k
    vpc = nblk * 128

    per_core = []
    counts = np.zeros((n_cores, nblk), np.int64)
    for c in range(n_cores):
        gb = send >> 7
        m = owner[gb] == c
        s_loc = kidx[gb[m]] * 128 + (send[m] & 127)
        r = recv[m]
        o = np.argsort(s_loc, kind="stable")
        s_loc, r = s_loc[o], r[o]
        blk = s_loc >> 7
        counts[c] = np.bincount(blk, minlength=nblk)
        per_core.append((s_loc, r, blk))

    qk = np.maximum(1, _cdiv(counts, 128).max(axis=0)).astype(np.int64)
    blk_g0 = np.zeros(nblk + 1, np.int64)
    blk_g0[1:] = np.cumsum(qk)
    Q = int(blk_g0[-1])
    QP = _cdiv(Q, cpb) * cpb
    n_batches = QP // cpb

    fpad = _cdiv(n_fac, 128) * 128
    zf_base = 32768 if fpad > 32767 else 0

    core_data = []
    for c in range(n_cores):
        s_loc, r, blk = per_core[c]
        n = s_loc.shape[0]
        blk_first = np.zeros(nblk, np.int64)
        blk_first[1:] = np.cumsum(counts[c])[:-1]
        pos = blk_g0[blk] * 128 + (np.arange(n) - blk_first[blk])

        slot_arr = np.full(QP * 128, SLOT_INVALID, np.float32)
        zidx_arr = np.zeros(QP * 128, np.int64)  # pads -> row zf_base
        slot_arr[pos] = (s_loc - blk * 128).astype(np.float32)
        zidx_arr[pos] = r - zf_base

        # every 1024-idx window must end with a non-negative zf index
        gs = min(1024, cpb * 128)
        for b in range(QP * 128 // gs):
            last = b * gs + gs - 1
            if zidx_arr[last] >= 0:
                continue
            chunk = slice(b * gs + gs - 128, b * gs + gs)
            cand = np.where(zidx_arr[chunk] >= 0)[0]
            assert cand.size > 0, "gather tail chunk has no non-negative zf idx"
            j = b * gs + gs - 128 + cand[-1]
            for arr in (slot_arr, zidx_arr):
                arr[last], arr[j] = arr[j], arr[last]

        slot_t = (
            slot_arr.reshape(n_batches, cpb, 128).transpose(2, 0, 1).reshape(128, QP)
        ).astype(BF16)
        slot_row = slot_arr[None, :].astype(BF16)

        w = (
            zidx_arr.reshape(n_batches, cpb * 8, 16)
            .transpose(2, 0, 1)
            .reshape(16, QP * 8)
        ).astype(np.int16)
        zf_idx = np.tile(w, (8, 1))

        core_data.append(dict(slot_t=slot_t, slot_row=slot_row, zf_idx=zf_idx))

    static = dict(
        vpc=vpc,
        nblk=nblk,
        qk=[int(x) for x in qk],
        blk_g0=[int(x) for x in blk_g0],
        Q=Q,
        QP=QP,
        cpb=cpb,
        n_batches=n_batches,
        vpad=nblk * 128,
        fpad=fpad,
        zf_base=zf_base,
        n_fac=n_fac,
        n_var=n_var,
        gblk=gblk,
        blocks_of_core=blocks_of_core,
    )
    return static, core_data


# --------------------------------------------------------------------------
# Bass program builder
# --------------------------------------------------------------------------

def _build_program(st):
    import concourse.mybir as mybir
    from concourse import bacc
    from concourse.tile import TileContext

    dt = mybir.dt
    f32, bf16, i16, u8 = dt.float32, dt.bfloat16, dt.int16, dt.uint8
    fp8 = dt.float8e4
    AF = mybir.ActivationFunctionType
    ALU = mybir.AluOpType
    DR = mybir.MatmulPerfMode.DoubleRow

    vpc, nblk = st["vpc"], st["nblk"]
    vpad, fpad = st["vpad"], st["fpad"]
    QP, cpb, n_batches = st["QP"], st["cpb"], st["n_batches"]
    qk, blk_g0 = st["qk"], st["blk_g0"]
    fblk = fpad // 128
    zf_base = st["zf_base"]

    nc = bacc.Bacc(
        None,
        target_bir_lowering=False,
        num_swdge_queues=4,
        dynamic_dma_scratch_size=32768,
    )

    p_vt = nc.declare_dram_parameter("vt_slice", [128, vpad], bf16, isOutput=False)
    p_vrows = nc.declare_dram_parameter("v_rows", [vpc, 128], bf16, isOutput=False)
    p_ft = nc.declare_dram_parameter("ft", [128, fpad], bf16, isOutput=False)
    p_wm_top = nc.declare_dram_parameter("wm_top", [128, 128], bf16, isOutput=False)
    p_wm_bot = nc.declare_dram_parameter("wm_bot", [128, 128], bf16, isOutput=False)
    p_wc_top = nc.declare_dram_parameter("wc_top", [128, 128], bf16, isOutput=False)
    p_wc_bot = nc.declare_dram_parameter("wc_bot", [128, 128], bf16, isOutput=False)
    p_bm = nc.declare_dram_parameter("bm_row", [1, 128], bf16, isOutput=False)
    p_bc = nc.declare_dram_parameter("bc_row", [1, 128], bf16, isOutput=False)
    p_ones = nc.declare_dram_parameter("ones_row", [1, 128], bf16, isOutput=False)
    p_iota4 = nc.declare_dram_parameter("iota4", [128, 2048], bf16, isOutput=False)
    p_iotac = nc.declare_dram_parameter("iota_col_rep", [128, 2048], bf16, isOutput=False)
    p_ident = nc.declare_dram_parameter("ident", [128, 128], bf16, isOutput=False)
    p_zidx = nc.declare_dram_parameter("zf_idx", [128, QP * 8], i16, isOutput=False)
    p_slot = nc.declare_dram_parameter("slot_t", [128, QP], bf16, isOutput=False)
    p_srow = nc.declare_dram_parameter("slot_row", [1, QP * 128], bf16, isOutput=False)
    p_out = nc.declare_dram_parameter("out", [vpc, 128], bf16, isOutput=True)

    zf_stage = nc.dram_tensor("zf_stage", [fblk, 128, 128], bf16)

    with TileContext(nc) as tc:
        with (
            tc.tile_pool(name="const", bufs=1) as cpool,
            tc.tile_pool(name="pro_ft", bufs=2) as ftpool,
            tc.tile_pool(name="pro_ps", bufs=3, space="PSUM") as propsum,
            tc.tile_pool(name="pro_st", bufs=2) as prost,
            tc.tile_pool(name="gbuf", bufs=10) as gpool,
            tc.tile_pool(name="sbc", bufs=3) as sbcpool,
            tc.tile_pool(name="gtt", bufs=3) as gttpool,
            tc.tile_pool(name="g4", bufs=6) as g4pool,
            tc.tile_pool(name="msb", bufs=3) as mspool,
            tc.tile_pool(name="mps", bufs=2, space="PSUM") as mppsum,
            tc.tile_pool(name="aggps", bufs=2, space="PSUM") as aggpsum,
            tc.tile_pool(name="aggt", bufs=3) as aggtpool,
            tc.tile_pool(name="hps", bufs=1, space="PSUM") as hpsum,
            tc.tile_pool(name="vrow", bufs=2) as vrowpool,
            tc.tile_pool(name="outb", bufs=2) as outpool,
        ):
            def load_const(name, param, shape, dtype):
                t = cpool.tile(shape, dtype, tag=name)
                nc.sync.dma_start(out=t[:], in_=param[:, :])
                return t

            wm_top_sb = load_const("wm_top", p_wm_top, [128, 128], bf16)
            wm_bot_sb = load_const("wm_bot", p_wm_bot, [128, 128], bf16)
            wc_top_sb = load_const("wc_top", p_wc_top, [128, 128], bf16)
            wc_bot_sb = load_const("wc_bot", p_wc_bot, [128, 128], bf16)
            ident_sb = load_const("ident", p_ident, [128, 128], bf16)
            bm_sb = load_const("bm_row", p_bm, [1, 128], bf16)
            bc_sb = load_const("bc_row", p_bc, [1, 128], bf16)
            ones_sb = load_const("ones_row", p_ones, [1, 128], bf16)
            vt_sb = load_const("vt_slice", p_vt, [128, vpad], bf16)
            idx_sb = load_const("zf_idx", p_zidx, [128, QP * 8], i16)
            slot_sb = load_const("slot_t", p_slot, [128, QP], bf16)
            iotac_sb = load_const("iota_col_rep", p_iotac, [128, 2048], bf16)

            iota4_sb = cpool.tile([128, 16, 128], bf16, tag="iota4")
            nc.sync.dma_start(out=iota4_sb[:], in_=p_iota4[:, :])

            bm4_sb = cpool.tile([128, 512], bf16, tag="bm4")
            for r in range(4):
                nc.sync.dma_start(
                    out=bm4_sb[:, r * 128 : (r + 1) * 128],
                    in_=p_bm[0:1, :].to_broadcast([128, 128]),
                )

            yv_sb = cpool.tile([128, vpad], bf16, tag="yv_sb")

            # ---- prologue: yv = V @ Wm_top + bm (own slice, [slot, feat]) ----
            for g4 in range(0, nblk, 4):
                nsub = min(4, nblk - g4)
                ps = propsum.tile([128, 512], f32, tag="props")
                for jj in range(nsub):
                    j = g4 + jj
                    sl = slice(jj * 128, (jj + 1) * 128)
                    nc.tensor.matmul(
                        out=ps[:, sl],
                        lhsT=vt_sb[:, j * 128 : (j + 1) * 128],
                        rhs=wm_top_sb[:],
                        start=True,
                        stop=True,
                    )
                nc.vector.tensor_tensor(
                    out=yv_sb[:, g4 * 128 : (g4 + nsub) * 128],
                    in0=ps[:, : nsub * 128],
                    in1=bm4_sb[:, : nsub * 128],
                    op=ALU.add,
                )

            # ---- prologue: zf = F @ Wm_bot (full table, row-major, DRAM) ----
            FSTREAM = 16
            for J in range(0, fblk, FSTREAM):
                nch = min(FSTREAM, fblk - J)
                ftt = ftpool.tile([128, FSTREAM * 128], bf16, tag="ft")
                nc.sync.dma_start(
                    out=ftt[:, : nch * 128], in_=p_ft[:, J * 128 : (J + nch) * 128]
                )
                stg = prost.tile([128, FSTREAM * 128], bf16, tag="prost")
                for g4 in range(0, nch, 4):
                    nsub = min(4, nch - g4)
                    ps = propsum.tile([128, 512], f32, tag="props")
                    for jj in range(nsub):
                        sl = slice(jj * 128, (jj + 1) * 128)
                        nc.tensor.matmul(
                            out=ps[:, sl],
                            lhsT=ftt[:, (g4 + jj) * 128 : (g4 + jj + 1) * 128],
                            rhs=wm_bot_sb[:],
                            start=True,
                            stop=True,
                        )
                    nc.vector.tensor_copy(
                        out=stg[:, g4 * 128 : (g4 + nsub) * 128],
                        in_=ps[:, : nsub * 128],
                    )
                nc.sync.dma_start(
                    out=zf_stage[J : J + nch, :, :].transpose([1, 0, 2]),
                    in_=stg[:, : nch * 128].rearrange("p (j f) -> p j f", j=nch),
                )

            # ---- edge phase ----
            blk_of_chunk = []
            for k in range(nblk):
                blk_of_chunk += [k] * qk[k]
            blk_of_chunk += [-1] * (QP - len(blk_of_chunk))

            agg_ps = None
            for b in range(n_batches):
                zb = gpool.tile([128, cpb, 128], bf16, tag="zbuf")
                nc.gpsimd.dma_gather(
                    out_ap=zb[:],
                    in_ap=zf_stage[zf_base // 128 :, :, :].rearrange(
                        "j p f -> (j p) f"
                    ),
                    idxs_ap=idx_sb[:, b * cpb * 8 : (b + 1) * cpb * 8],
                    num_idxs=cpb * 128,
                    num_idxs_reg=cpb * 128,
                    elem_size=128,
                    single_packet=False,
                    queue_num=b % 4,
                )
                # slot stream broadcast to 128 partitions (DMA), then G^T
                sbc = sbcpool.tile([128, cpb * 128], bf16, tag="sbc")
                nc.sync.dma_start(
                    out=sbc[:],
                    in_=p_srow[0:1, b * cpb * 128 : (b + 1) * cpb * 128].to_broadcast(
                        [128, cpb * 128]
                    ),
                )
                gt_t = gttpool.tile([128, cpb * 128], bf16, tag="gtt")
                nc.vector.tensor_tensor(
                    out=gt_t[:], in0=sbc[:], in1=iotac_sb[:, : cpb * 128],
                    op=ALU.is_equal,
                )

                g16t = g4pool.tile([128, 16, 128], bf16, tag="g4")
                nc.vector.tensor_tensor(
                    out=g16t[:],
                    in0=slot_sb[:, b * cpb : (b + 1) * cpb].to_broadcast(
                        [128, 16, 128]
                    ),
                    in1=iota4_sb[:],
                    op=ALU.is_equal,
                )
                for g in range(cpb // 4):
                    g0 = b * cpb + g * 4  # first chunk of this 4-chunk group
                    m_ps = mppsum.tile([128, 512], f32, tag="mps")
                    nc.tensor.matmul(
                        out=m_ps[:],
                        lhsT=ident_sb[:],
                        rhs=zb[:, g * 4 : g * 4 + 4, :],
                        start=True,
                        stop=False,
                        skip_group_check=True,
                    )
                    for cc in range(4):
                        gch = g0 + cc
                        k = blk_of_chunk[gch]
                        kk = k if k >= 0 else 0
                        sl = slice(cc * 128, (cc + 1) * 128)
                        nc.tensor.matmul(
                            out=m_ps[:, sl],
                            lhsT=gt_t[:, (g * 4 + cc) * 128 : (g * 4 + cc + 1) * 128],
                            rhs=yv_sb[:, kk * 128 : (kk + 1) * 128],
                            start=False,
                            stop=(cc == 3),
                            skip_group_check=True,
                        )
                    msg_sb = mspool.tile([128, 512], bf16, tag="msb")
                    nc.scalar.activation(out=msg_sb[:], in_=m_ps[:], func=AF.Relu)

                    for cc in range(4):
                        gch = g0 + cc
                        k = blk_of_chunk[gch]
                        if k < 0:
                            continue
                        first = gch == blk_g0[k]
                        last = gch == blk_g0[k + 1] - 1
                        if first:
                            agg_ps = aggpsum.tile([128, 128], f32, tag="aggps")
                        nc.tensor.matmul(
                            out=agg_ps[:],
                            lhsT=msg_sb[:, cc * 128 : (cc + 1) * 128],
                            rhs=g16t[:, g * 4 + cc, :],
                            start=first,
                            stop=last,
                        )
                        if last:
                            vwid = min(128, vpc - k * 128)
                            aggt = aggtpool.tile([128, 128], bf16, tag="aggt")
                            nc.scalar.copy(out=aggt[:], in_=agg_ps[:])
                            h_ps = hpsum.tile([128, 128], f32, tag="hps")
                            nc.tensor.matmul(
                                out=h_ps[:vwid, :],
                                lhsT=vt_sb[:, k * 128 : k * 128 + vwid],
                                rhs=wc_top_sb[:],
                                start=True,
                                stop=False,
                            )
                            nc.tensor.matmul(
                                out=h_ps[:vwid, :],
                                lhsT=aggt[:, :vwid],
                                rhs=wc_bot_sb[:],
                                start=False,
                                stop=False,
                            )
                            nc.tensor.matmul(
                                out=h_ps[:vwid, :],
                                lhsT=ones_sb[:, :vwid],
                                rhs=bc_sb[:],
                                start=False,
                                stop=True,
                            )
                            vt_in = vrowpool.tile([128, 128], bf16, tag="vrow")
                            nc.sync.dma_start(
                                out=vt_in[:vwid, :],
                                in_=p_vrows[k * 128 : k * 128 + vwid, :],
                            )
                            ot = outpool.tile([128, 128], bf16, tag="outb")
                            nc.vector.scalar_tensor_tensor(
                                out=ot[:vwid, :],
                                in0=h_ps[:vwid, :],
                                scalar=0.0,
                                in1=vt_in[:vwid, :],
                                op0=ALU.max,
                                op1=ALU.add,
                            )
                            nc.sync.dma_start(
                                out=p_out[k * 128 : k * 128 + vwid, :],
                                in_=ot[:vwid, :],
                            )

    nc.finalize()
    return nc


# --------------------------------------------------------------------------
# Host-side input preparation
# --------------------------------------------------------------------------

def _make_in_maps(variables, factors, Wm, bm, Wc, bc, st, core_data):
    vpc, vpad, fpad = st["vpc"], st["vpad"], st["fpad"]
    n_cores = len(core_data)

    V = np.asarray(variables, dtype=np.float32)
    F = np.asarray(factors, dtype=np.float32)
    Wm = np.asarray(Wm, dtype=np.float32)
    Wc = np.asarray(Wc, dtype=np.float32)
    bm = np.asarray(bm, dtype=np.float32)
    bc = np.asarray(bc, dtype=np.float32)

    ftp = np.zeros((128, fpad), dtype=BF16)
    ftp[:, : F.shape[0]] = F.T.astype(BF16)

    iota = np.arange(128, dtype=np.float32)
    shared = dict(
        ft=ftp,
        wm_top=Wm[:128, :].astype(BF16),
        wm_bot=Wm[128:, :].astype(BF16),
        wc_top=Wc[:128, :].astype(BF16),
        wc_bot=Wc[128:, :].astype(BF16),
        bm_row=bm[None, :].astype(BF16),
        bc_row=bc[None, :].astype(BF16),
        ones_row=np.ones((1, 128), dtype=BF16),
        ident=np.eye(128, dtype=np.float32).astype(BF16),
        iota4=np.tile(iota[None, :], (128, 16)).astype(BF16),
        iota_col_rep=np.tile(
            np.arange(128, dtype=np.float32)[:, None], (1, 2048)
        ).astype(BF16),
    )

    boc = st["blocks_of_core"]
    n_var = st["n_var"]
    in_maps = []
    for c in range(n_cores):
        vslice = np.zeros((vpc, 128), dtype=np.float32)
        for k in range(st["nblk"]):
            g = boc[c, k]
            if g < 0:
                continue
            lo = g * 128
            w = min(128, n_var - lo)
            vslice[k * 128 : k * 128 + w] = V[lo : lo + w]
        m = dict(shared)
        m["vt_slice"] = np.ascontiguousarray(vslice.T).astype(BF16)
        m["v_rows"] = vslice.astype(BF16)
        m["slot_t"] = core_data[c]["slot_t"]
        m["slot_row"] = core_data[c]["slot_row"]
        m["zf_idx"] = core_data[c]["zf_idx"]
        in_maps.append(m)
    return in_maps


# --------------------------------------------------------------------------
# Public entry point
# --------------------------------------------------------------------------

def kernel(variables, factors, senders, receivers, Wm, bm, Wc, bc, _trace=False):
    from concourse.bass_utils import run_bass_kernel_spmd

    st, core_data = _make_plan(senders, receivers, N_VAR, N_FAC, N_CORES, CPB)
    nc = _build_program(st)
    in_maps = _make_in_maps(variables, factors, Wm, bm, Wc, bc, st, core_data)
    res = run_bass_kernel_spmd(
        nc, in_maps, core_ids=list(range(N_CORES)), trace=_trace
    )
    out = np.empty((N_VAR, 128), dtype=np.float32)
    boc = st["blocks_of_core"]
    for c in range(N_CORES):
        oc = np.asarray(res.results[c]["out"], dtype=np.float32)
        for k in range(st["nblk"]):
            g = boc[c, k]
            if g < 0:
                continue
            lo = g * 128
            w = min(128, N_VAR - lo)
            out[lo : lo + w] = oc[k * 128 : k * 128 + w]
    if _trace:
        kernel.last_exec_time_ns = res.exec_time_ns
        kernel.last_results = res
    return out
